# revision 65
# baseline (speedup 1.0000x reference)
"""Trainium2 Bass kernel for nn_CausalAttention_5815385719336.

Dual-softmax attention: out = softmax(-QK^T/8) V Wo^T (+bias folds),
out_comp = softmax(+QK^T/8) V Wo^T.  B=2, S=2048, D=1024, H=16, DK=64.

Sharding (8 cores): Megatron-style head parallel.  Core c owns heads
(2c, 2c+1) = output dims [128c, 128c+128) of the QKV projections.  Each
core computes its head slice of Q/K/V for both batches, the full [S,S]
attention for its 4 (b, head) units (both softmax branches), and a
partial output projection o_slice @ Wo_slice^T.  The host sums the 8
partial outputs and adds the bias fold (bv @ Wo^T + bo).

On-device dataflow is fully "transposed": the host ships x^T (and W^T
slices) so every matmul contracts along partitions with zero on-device
transposes.  Scores are built as scores^T [k, q]; exp runs on the
scalar engine straight out of PSUM; P^T @ V needs no transpose.

The exp stream is the roofline.  Four of each unit's 16 FD-1024 exp
instructions run on the DVE as a Schraudolph bit-trick
(int16(A*s+B) bitcast to bf16, C tuned so the ~1.8% rms log error is
zero-mean; both branches see the same noise), cutting the scalar
engine from ~291us to ~225us of ACTIVATEs.  Scheduling:
 - score tiles are [128,2,512] PSUM pairs in a 2-deep ring; one FD-1024
   exp covers a kt pair so ACT pays its ~300-cycle overhead half as
   often;
 - a kernel-wide software pipeline (pv_queue) runs every PV/pending
   matmul two segments behind its scores/exps, flowing across unit
   boundaries: all PE work after a segment's two score matmuls is
   ready when it issues, so feed/outproj lumps amortize instead of
   starving ACT;
 - PV accumulators ([65,512], denominators riding as ones-columns of
   V) allocate lazily inside the pipeline closures so the 2-bank ring
   sequences each alloc right after the copy that frees its
   predecessor;
 - batch-0's projection feed runs up front; batch-1's entire feed
   spreads across b0's mid/late units as bg markers, doubling as the
   PE-duty filler that keeps the HAM at full clock; explicit warm
   fillers (targets chosen so they never wait a cross-engine
   semaphore) cover the b1 half and the tail;
 - the final unit runs its second branch eagerly and the tail outproj
   half-chunks borrow the idle score ring, leaving only two norm
   chains + 16 half-chunks after the last exp.
"""

import numpy as np
import ml_dtypes

B, S, D, H, DK = 2, 2048, 1024, 16, 64
NCORES = 8
HPC = H // NCORES          # heads per core = 2
DSL = HPC * DK             # d-slice per core = 128
P = 128
BF16 = ml_dtypes.bfloat16

_compiled = {}


def _install_drain_split():
    """walrus in this container rejects >1 sync wait on the Tile tail
    Drain; split extra waits into standalone wait_ge instructions."""
    import concourse.tile as tile
    from concourse.vector_clock import ScopedClock

    if getattr(tile.TileContext, "_drain_split_installed", False):
        return

    def _drain_and_barrier(self, tick_clock, wait_clock):
        nc = self.nc
        drain_inst = nc.sync.drain()
        wait_clock.add_sem_waits(
            drain_inst.ins, ScopedClock({None: tick_clock.global_clock})
        )
        si = drain_inst.ins.sync_info
        if si is not None and si.on_wait and len(si.on_wait) > 1:
            waits = list(si.on_wait)
            handles = {h.num: h for h in self.sems.allocated().values()}
            si.on_wait = waits[:1]
            for w in waits[1:]:
                assert w.wait_mode == "sem-ge-imm", w.wait_mode
                nc.sync.wait_ge(handles[w.id], w.wait_value)
        nc.all_engine_barrier()
        popped = nc._tile_sem_poison_stack.pop()
        assert popped is self._sem_poison
        nc.clear_and_free_semaphores(list(self.sems.allocated().values()))
        nc.all_engine_barrier()

    tile.TileContext._drain_and_barrier = _drain_and_barrier
    tile.TileContext._drain_split_installed = True


def _split_sync_waits(nc, max_waits=1):
    """walrus in this container has a small per-instruction sync-wait
    capacity.  Hoist excess waits onto standalone EventSemaphore
    instructions inserted just before the owner on the same engine —
    program order within an engine keeps the semantics identical."""
    from concourse import mybir

    n = 0
    for bb in nc.main_func.blocks:
        out = []
        for ins in bb.instructions:
            si = ins.sync_info
            if si is not None and si.on_wait and len(si.on_wait) > max_waits:
                waits = list(si.on_wait)
                for w in waits[:-max_waits]:
                    wi = mybir.InstEventSemaphore(name=f"W-split-{n}", ins=[], outs=[])
                    n += 1
                    wi.engine = ins.engine
                    wi.sync_info = mybir.SyncInfo(on_wait=[w], on_update=[])
                    out.append(wi)
                si.on_wait = waits[-max_waits:]
            out.append(ins)
        if n:
            bb.instructions = out


def _build():
    import concourse.bass as bass
    import concourse.tile as tile
    from concourse import mybir

    _install_drain_split()

    f32 = mybir.dt.float32
    bf16 = mybir.dt.bfloat16
    i16 = mybir.dt.int16
    Exp = mybir.ActivationFunctionType.Exp
    NT = B * S                      # 4096 tokens
    ET = D // P                     # 8 e-tiles
    # Schraudolph bit-trick exp on the DVE: int16(SCHRA*s + SCHRB)
    # reinterpreted as bf16 equals exp(s/8)*(1+eps), |eps|<~5%, zero-mean
    # (C=7.3 centers the log error; HW convert is round-to-nearest,
    # verified by probe).  Offloading 2 of 16 exp instructions per
    # attention unit to the idle DVE shaves the ACT roofline.
    SCHRA = float(0.125 * (2.0**7) / np.log(2.0))
    SCHRB = float(127 * 2**7 - 7.3)
    AluMult = mybir.AluOpType.mult
    AluAdd = mybir.AluOpType.add

    nc = bass.Bass()
    NCH = NT // 512            # 8 token chunks of 512
    xt_d = nc.declare_dram_parameter("xt", [NCH, P, ET, 512], bf16, isOutput=False)
    wq_d = nc.declare_dram_parameter("wq", [P, ET, DSL], bf16, isOutput=False)
    wk_d = nc.declare_dram_parameter("wk", [P, ET, DSL], bf16, isOutput=False)
    wv_d = nc.declare_dram_parameter("wv", [P, ET, DSL], bf16, isOutput=False)
    wo_d = nc.declare_dram_parameter("wo", [P, D], bf16, isOutput=False)
    bqk_d = nc.declare_dram_parameter("bqk", [P, 2], f32, isOutput=False)
    out_d = nc.declare_dram_parameter("out", [2, B, S, D], bf16, isOutput=True)

    KT = S // P                     # 16 k-tiles per batch
    TT = S // P                     # 16 token-tiles per batch
    QC = 4                          # q chunks per batch
    QW = S // QC                    # 512

    with tile.TileContext(nc) as tc:
        with (
            tc.tile_pool(name="singles", bufs=1) as singles,
            tc.tile_pool(name="xst", bufs=4) as xst,
            tc.tile_pool(name="perb", bufs=2) as perb,
            tc.tile_pool(name="stash", bufs=16) as stash,
            # 8-deep ex ring: a 4-deep ring made ACT exps inherit WAW
            # waits on delayed DVE Schraudolph writes when reclaiming a
            # buffer; 8 kps of distance decouples the engines fully
            tc.tile_pool(name="expp", bufs=8) as expp,
            tc.tile_pool(name="otsp", bufs=2) as otsp,
            tc.tile_pool(name="normp", bufs=4) as normp,
            tc.tile_pool(name="outp", bufs=4) as outp,
            # 8 PSUM banks: ps_sc 2x[128,2,512] = 4 (score kt-pair ring,
            # nothing else ever allocates here; one FD=1024 exp covers a
            # pair so ACT pays its ~290ns per-instruction overhead half
            # as often), ps_acc 2x[128,512] = 2 (PV pos + pending neg),
            # ps_chunk 2x[128,512] = 2 (projection / outproj /
            # denominator-broadcast chunks).
            tc.tile_pool(name="ps_sc", bufs=2, space="PSUM") as ps_sc,
            tc.tile_pool(name="ps_acc", bufs=2, space="PSUM") as ps_acc,
            tc.tile_pool(name="ps_chunk", bufs=2, space="PSUM") as ps_chunk,
        ):
            # weight tiles are allocated here but their DMAs ride inside
            # the projection feed, AFTER x0 — the 1MB x chunk is the
            # first-score critical path, the weights are small.
            wk = singles.tile([P, ET, DSL], bf16)
            wq = singles.tile([P, ET, DSL], bf16)
            bqk = singles.tile([P, 2], f32)
            bq = bqk[:, 0:1]
            bk = bqk[:, 1:2]
            wv = singles.tile([P, ET, DSL], bf16)
            wo = singles.tile([P, D], bf16)
            warm = singles.tile([P, 512], bf16)
            nc.gpsimd.memset(warm[:], 0.0)
            ones_sb = singles.tile([P, 64], bf16)
            nc.vector.memset(ones_sb[:], 1.0)

            def load_wk():
                # weight DMAs are ordered so the first-score critical
                # path (x0+wk+biases, then wq) owns the HBM stream and
                # the Sync engine's ~0.65us-per-push budget
                nc.sync.dma_start(wk[:], wk_d[:])
                nc.sync.dma_start(bqk[:], bqk_d[:])

            def load_wq():
                nc.sync.dma_start(wq[:], wq_d[:])

            def load_wv_wo():
                nc.sync.dma_start(wv[:], wv_d[:])
                nc.sync.dma_start(wo[:], wo_d[:])

            def warmup_pe():
                # ~3.5us of throwaway matmuls bridging the initial x DMA
                # wait: trips the PE HAM to K=8/8 (2.4 GHz, needs ~3.4us of
                # sustained busy) so the first projection matmuls run warm.
                ps_w = ps_chunk.tile([P, 512], f32, tag="chunk", name="warm")
                for _ in range(15):
                    nc.tensor.matmul(ps_w[:, 0:256], warm[:, 0:128],
                                     warm[:, 0:256], start=True, stop=True)

            fill_state = {}

            def filler_into(ap, n):
                # HAM-warming throwaway matmuls.  The target must be a
                # psum region whose readers have already retired (the
                # previous kp's exp'd score tile, or a dead acc bank):
                # a filler that waits on a cross-engine semaphore stalls
                # the in-order PE stream and starves ACT instead of
                # helping it.
                for _ in range(n):
                    nc.tensor.matmul(ap, warm[:, 0:128], warm[:],
                                     start=True, stop=True)

            # ---------- background queues ----------
            bg_queue = []

            def drain_bg(n=1):
                for _ in range(n):
                    if not bg_queue:
                        return
                    bg_queue.pop(0)()

            norm_queue = []

            def drain_norm(n=1):
                for _ in range(n):
                    if not norm_queue:
                        return
                    norm_queue.pop(0)()

            pending = {}

            def emit_pending_mms(k0, k1):
                if not pending:
                    return
                exn, accn, vtp, vlo, vhi = (pending[k] for k in
                                            ("exn", "acc", "vt", "vlo",
                                             "vhi"))
                for kt in range(k0, k1):
                    nc.tensor.matmul(
                        accn[0:65, :],
                        vtp[:, kt, vlo:vhi],
                        exn[kt // 2][:, kt % 2, :],
                        start=(kt == 0),
                        stop=(kt == KT - 1),
                    )

            def defer_norm(acc, b, oTs, br, h, q0, name):
                """Copy acc out of PSUM now (f32, frees its bank),
                reciprocal the denominator row on the DVE now (spread
                across 64 lanes via a tiny DMA reshape), and defer the
                ones-matmul broadcast + multiply into norm_queue.
                head1's V rows shift to partitions 64:128 via a small
                DMA so the output projection contracts both heads in
                one matmul.  The h==1 stage also emits output-projection
                half-chunks once its oTs columns are complete."""
                hp = 64 * h
                oTuD = normp.tile([P, QW], f32, tag="oTuD", name=f"oTuD{name}")
                nc.vector.tensor_copy(oTuD[0:65, :], acc[0:65, :])
                rsh = normp.tile([64, 8], f32, tag="rsh", name=f"rsh{name}")
                nc.sync.dma_start(rsh[:], oTuD[64:65, :])
                rshr = normp.tile([64, 8], bf16, tag="rshr", name=f"rshr{name}")
                with nc.allow_low_precision(reason="1/denom to bf16"):
                    nc.vector.reciprocal(rshr[:], rsh[:])
                rcp = normp.tile([P, QW], bf16, tag="rcp", name=f"rcp{name}")
                nc.sync.dma_start(rcp[64:65, :], rshr[:])
                if h == 1:
                    oTuD2 = normp.tile([P, QW], f32, tag="oTuD2",
                                       name=f"oTuD2{name}")
                    nc.sync.dma_start(oTuD2[64:128, :], oTuD[0:64, :])
                    oTuD = oTuD2

                def stage():
                    bc = ps_chunk.tile([P, QW], f32, tag="chunk",
                                       name=f"bc{name}")
                    nc.tensor.matmul(
                        bc[hp : hp + 64, :],
                        ones_sb[64:65, :],
                        rcp[64:65, :],
                        start=True,
                        stop=True,
                    )
                    nc.vector.tensor_mul(
                        oTs[br][hp : hp + 64, q0 : q0 + QW],
                        oTuD[hp : hp + 64, :],
                        bc[hp : hp + 64, :],
                    )
                    if h == 1:
                        qc = q0 // QW
                        bg_queue.extend(
                            outproj_chunks(b, oTs, br,
                                           range(qc * 4, qc * 4 + 4),
                                           alt=(b == 1 and qc == QC - 1))
                        )

                norm_queue.append(stage)

            def finish_pending():
                if not pending:
                    return
                defer_norm(pending["acc"], pending["b"], pending["oTs"], 1,
                           pending["h"], pending["q0"], pending["name"] + "n")
                pending.clear()

            # ---------- projections (progressive feed) ----------
            feeds = {}
            feed_state = {}

            def make_feed(b):
                t0 = b * S
                qT = perb.tile([P, S], bf16, tag="qT", name=f"qT_{b}")
                kT = perb.tile([P, S], bf16, tag="kT", name=f"kT_{b}")
                # vt columns: 0:64 = V head0, 64 = ones (head0 denom),
                # 65:129 = V head1, 129 = ones (head1 denom); each
                # head's PV stationary is a 65-col slice -> acc rows
                # 0:65 with the denominator at row 64.
                vt = perb.tile([P, TT, 130], bf16, tag="vt", name=f"vt_{b}")
                nc.vector.memset(vt[:, :, 64], 1.0)
                nc.vector.memset(vt[:, :, 129], 1.0)
                cell = {}

                def load_chunk(xc, split=False):
                    def go():
                        xtile = xst.tile([P, ET, 512], bf16, tag="xtile",
                                         name=f"xt_{b}_{xc}")
                        gc = b * QC + xc
                        if split:
                            # two parallel DMAs halve the critical-path
                            # latency of the very first x chunk
                            nc.sync.dma_start(xtile[:, 0:4, :],
                                              xt_d[gc, :, 0:4, :])
                            nc.sync.dma_start(xtile[:, 4:8, :],
                                              xt_d[gc, :, 4:8, :])
                        else:
                            nc.sync.dma_start(xtile[:], xt_d[gc])
                        cell[xc] = xtile
                    return go

                def qk_chunk(xc, w_t, bias_t, dst, half):
                    # half 0 emits the first 4 e-tiles, half 1 the last 4
                    # plus the bias/copy-out: each feed pop stays <1us of
                    # PE so attention score matmuls never wait long.
                    def go():
                        xtile = cell[xc]
                        key = ("pj", xc, id(w_t))
                        if half == 0:
                            ps = ps_chunk.tile([P, 512], f32, tag="chunk",
                                               name=f"pj_{b}_{xc}_{id(w_t)}")
                            cell[key] = ps
                        else:
                            ps = cell[key]
                        for et in range(4 * half, 4 * half + 4):
                            nc.tensor.matmul(
                                ps, w_t[:, et, :], xtile[:, et, :],
                                start=(et == 0), stop=(et == ET - 1),
                            )
                        if half == 1:
                            nc.vector.tensor_scalar_add(
                                dst[:, xc * 512 : (xc + 1) * 512], ps, bias_t
                            )
                    return go

                def v_chunk(xc, vtt):
                    def go():
                        xtile = cell[xc]
                        tt = xc * 4 + vtt
                        pv = ps_chunk.tile([P, DSL], f32, tag="chunk",
                                           name=f"pv_{b}_{tt}")
                        for et in range(ET):
                            nc.tensor.matmul(
                                pv, xtile[:, et, vtt * P : (vtt + 1) * P],
                                wv[:, et, :],
                                start=(et == 0), stop=(et == ET - 1),
                            )
                        nc.vector.tensor_copy(vt[:, tt, 0:64], pv[:, 0:64])
                        nc.vector.tensor_copy(vt[:, tt, 65:129],
                                              pv[:, 64:128])
                    return go

                if b == 0:
                    items = [
                        ("x", 0, load_chunk(0, split=True)),
                        ("w", 0, load_wk),
                        ("kTh", 0, qk_chunk(0, wk, bk, kT, 0)),
                        ("kT", 0, qk_chunk(0, wk, bk, kT, 1)),
                        ("w", 1, load_wq),
                        ("qTh", 0, qk_chunk(0, wq, bq, qT, 0)),
                        ("qT", 0, qk_chunk(0, wq, bq, qT, 1)),
                        ("w", 2, load_wv_wo),
                        ("x", 1, load_chunk(1)),
                        ("x", 2, load_chunk(2)),
                        ("x", 3, load_chunk(3)),
                        ("v", 0, v_chunk(0, 0)),
                        ("v", 1, v_chunk(0, 1)),
                        ("v", 2, v_chunk(0, 2)),
                        ("v", 3, v_chunk(0, 3)),
                        ("kTh", 1, qk_chunk(1, wk, bk, kT, 0)),
                        ("kT", 1, qk_chunk(1, wk, bk, kT, 1)),
                        ("qTh", 1, qk_chunk(1, wq, bq, qT, 0)),
                        ("qT", 1, qk_chunk(1, wq, bq, qT, 1)),
                    ]
                else:
                    # batch 1: all x loads issue up front (the DMAs
                    # overlap late-b0 compute; xst is 4 deep)
                    items = [
                        ("x", 0, load_chunk(0)),
                        ("x", 1, load_chunk(1)),
                        ("x", 2, load_chunk(2)),
                        ("x", 3, load_chunk(3)),
                        ("kTh", 0, qk_chunk(0, wk, bk, kT, 0)),
                        ("kT", 0, qk_chunk(0, wk, bk, kT, 1)),
                        ("qTh", 0, qk_chunk(0, wq, bq, qT, 0)),
                        ("qT", 0, qk_chunk(0, wq, bq, qT, 1)),
                        ("v", 0, v_chunk(0, 0)),
                        ("v", 1, v_chunk(0, 1)),
                        ("v", 2, v_chunk(0, 2)),
                        ("v", 3, v_chunk(0, 3)),
                        ("kTh", 1, qk_chunk(1, wk, bk, kT, 0)),
                        ("kT", 1, qk_chunk(1, wk, bk, kT, 1)),
                        ("qTh", 1, qk_chunk(1, wq, bq, qT, 0)),
                        ("qT", 1, qk_chunk(1, wq, bq, qT, 1)),
                    ]
                items += [
                    ("v", 4, v_chunk(1, 0)),
                    ("v", 5, v_chunk(1, 1)),
                    ("v", 6, v_chunk(1, 2)),
                    ("v", 7, v_chunk(1, 3)),
                    ("kTh", 2, qk_chunk(2, wk, bk, kT, 0)),
                    ("kT", 2, qk_chunk(2, wk, bk, kT, 1)),
                    ("qTh", 2, qk_chunk(2, wq, bq, qT, 0)),
                    ("qT", 2, qk_chunk(2, wq, bq, qT, 1)),
                    ("v", 8, v_chunk(2, 0)),
                    ("v", 9, v_chunk(2, 1)),
                    ("v", 10, v_chunk(2, 2)),
                    ("v", 11, v_chunk(2, 3)),
                    ("kTh", 3, qk_chunk(3, wk, bk, kT, 0)),
                    ("kT", 3, qk_chunk(3, wk, bk, kT, 1)),
                    ("qTh", 3, qk_chunk(3, wq, bq, qT, 0)),
                    ("qT", 3, qk_chunk(3, wq, bq, qT, 1)),
                    ("v", 12, v_chunk(3, 0)),
                    ("v", 13, v_chunk(3, 1)),
                    ("v", 14, v_chunk(3, 2)),
                    ("v", 15, v_chunk(3, 3)),
                ]
                feeds[b] = items
                feed_state[b] = {"x": -1, "kT": -1, "qT": -1, "v": -1,
                                 "w": -1, "kTh": -1, "qTh": -1}
                return qT, kT, vt

            def feed_pop(b):
                if not feeds[b]:
                    return False
                label, idx, go = feeds[b].pop(0)
                go()
                feed_state[b][label] = idx
                return True

            def require(b, label, idx):
                while feed_state[b][label] < idx:
                    assert feed_pop(b), (b, label, idx)

            def feed_marker(b):
                def go():
                    feed_pop(b)
                return go

            def outproj_chunks(b, oTs, br, tts, alt=False):
                """Output-projection half-chunk closures for (b, br,
                tts): one [128,512] matmul + cast + DMA per (tt, oc).
                alt=True marks tail chunks: casts alternate between DVE
                and ACT (ACT has gone idle), and the po tiles come from
                the score-ring PSUM pool (idle at the tail) so several
                matmuls can be in flight ahead of the casts."""
                chunks = []
                cellp = {}

                def one(tt, oc, cast_eng):
                    def go():
                        if alt:
                            if tt not in cellp:
                                cellp[tt] = ps_sc.tile(
                                    [P, 2, QW], f32, tag="sc",
                                    name=f"pot_{b}_{br}_{tt}")
                            po = cellp[tt][:, oc, :]
                        else:
                            po = ps_chunk.tile([P, 512], f32, tag="chunk",
                                               name=f"po_{b}_{br}_{tt}_{oc}")
                        nc.tensor.matmul(
                            po,
                            oTs[br][:, tt * P : (tt + 1) * P],
                            wo[:, oc * 512 : (oc + 1) * 512],
                            start=True,
                            stop=True,
                        )
                        ob = outp.tile([P, 512], bf16, tag="ob")
                        cast_eng(ob[:], po)
                        nc.sync.dma_start(
                            out_d[br, b, tt * P : (tt + 1) * P,
                                  oc * 512 : (oc + 1) * 512], ob[:]
                        )
                    return go

                i = 0
                for tt in tts:
                    for oc in range(2):
                        # casts alternate ACT/DVE everywhere: with four
                        # exps per unit offloaded to the DVE, ACT has
                        # exactly the slack for half the casts — keeping
                        # its queue full instead of idling, while the
                        # lighter DVE queue returns the Schraudolph
                        # exps well inside the score-ring slack.
                        eng = (nc.scalar.copy if (alt and i % 2 == 1)
                               else nc.vector.tensor_copy)
                        chunks.append(one(tt, oc, eng))
                        i += 1
                return chunks

            # ---------- attention ----------
            # The kernel-wide software pipeline: scores+exps for a kp
            # issue first; the matching PV/pending matmuls are queued as
            # closures and pop TWO segments later (flowing across unit
            # boundaries), so every PE matmul after the two score mms at
            # a segment's head is ready the moment it issues — feed and
            # outproj lumps amortize inside the two-segment slack instead
            # of delaying the next score pair and starving ACT.
            unit_ctr = [0]
            pv_queue = []

            def pv_pump(limit=2):
                while len(pv_queue) > limit:
                    pv_queue.pop(0)()

            def attention(b, qT, kT, vt, oTs, inject=None, last=False):
                # qc outer / h inner: each (h1, qc) norm completes a
                # quarter-batch of output columns mid-attention, so its
                # output-projection halves drain before the tail.
                prev_sc = [None]
                for qc in range(QC):
                    q0 = qc * QW
                    for h in range(HPC):
                        hp = 64 * h
                        vlo, vhi = (0, 65) if h == 0 else (65, 130)
                        require(b, "qT", qc)
                        name = f"_{b}_{h}_{qc}"
                        is_last = last and qc == QC - 1 and h == HPC - 1
                        uidx = unit_ctr[0]
                        unit_ctr[0] += 1
                        # per-kp exn tiles: the deferred branch's exps
                        # come from two engines (ACT and the DVE bit
                        # trick); separate tiles keep each one single-
                        # writer so no cross-engine write-order coupling
                        # ever delays the exp stream
                        exn = [stash.tile([P, 2, QW], bf16, tag="exn",
                                          name=f"exn{name}_{kp}")
                               for kp in range(KT // 2)]
                        # acc banks allocate lazily inside the closures:
                        # the 2-deep psum ring only works because each
                        # accumulator is claimed right after the copy
                        # that frees its predecessor has been emitted.
                        ust = {"acc": None, "accn": None}

                        def mk_pv(kp, ex, ust=ust, vt=vt, vlo=vlo,
                                  vhi=vhi, b=b, is_last=is_last, exn=exn,
                                  name=name):
                            def go():
                                if ust["acc"] is None:
                                    ust["acc"] = ps_acc.tile(
                                        [P, QW], f32, tag="acc",
                                        name=f"accp{name}")
                                acc = ust["acc"]
                                require(b, "v", 2 * kp + 1)
                                for j in range(2):
                                    kt = 2 * kp + j
                                    nc.tensor.matmul(
                                        acc[0:65, :],
                                        vt[:, kt, vlo:vhi],
                                        ex[:, j, :],
                                        start=(kt == 0),
                                        stop=(kt == KT - 1),
                                    )
                                    if not is_last:
                                        emit_pending_mms(kt, kt + 1)
                                if is_last:
                                    # final unit: drain the previous
                                    # unit's deferred branch at 4 mms per
                                    # segment so its acc bank frees
                                    # mid-loop, then accumulate this
                                    # unit's own second branch eagerly.
                                    if kp < 4:
                                        emit_pending_mms(4 * kp, 4 * kp + 4)
                                        if kp == 3:
                                            finish_pending()
                                    else:
                                        if ust["accn"] is None:
                                            ust["accn"] = ps_acc.tile(
                                                [P, QW], f32, tag="acc",
                                                name=f"accn{name}")
                                        lo, hi = {4: (0, 6), 5: (6, 10),
                                                  6: (10, 14),
                                                  7: (14, 16)}[kp]
                                        for kte in range(lo, hi):
                                            nc.tensor.matmul(
                                                ust["accn"][0:65, :],
                                                vt[:, kte, vlo:vhi],
                                                exn[kte // 2][:, kte % 2, :],
                                                start=(kte == 0),
                                                stop=(kte == KT - 1),
                                            )
                            return go

                        def mk_finish(ust=ust, b=b, oTs=oTs, h=h, q0=q0,
                                      name=name, is_last=is_last, vt=vt,
                                      vlo=vlo, vhi=vhi, exn=exn):
                            def go():
                                if is_last:
                                    defer_norm(ust["acc"], b, oTs, 0, h,
                                               q0, name + "p")
                                    defer_norm(ust["accn"], b, oTs, 1, h,
                                               q0, name + "n")
                                    tail_fill = ps_acc.tile(
                                        [P, QW], f32, tag="acc",
                                        name="tailfill")
                                    fill_state["tail"] = tail_fill
                                    filler_into(tail_fill[:], 10)
                                    return
                                finish_pending()
                                defer_norm(ust["acc"], b, oTs, 0, h, q0,
                                           name + "p")
                                accn = ps_acc.tile([P, QW], f32, tag="acc",
                                                   name=f"accn{name}")
                                pending.update(exn=exn, acc=accn, vt=vt,
                                               vlo=vlo, vhi=vhi, oTs=oTs,
                                               b=b, h=h, q0=q0, name=name)
                            return go

                        for kp in range(KT // 2):
                            k0 = 2 * kp
                            require(b, "kT", (k0 + 1) // 4)
                            sc = ps_sc.tile([P, 2, QW], f32, tag="sc",
                                            name=f"sc{name}_{kp}")
                            for j in range(2):
                                kt = k0 + j
                                nc.tensor.matmul(
                                    sc[:, j, :],
                                    kT[hp : hp + 64, kt * P : (kt + 1) * P],
                                    qT[hp : hp + 64, q0 : q0 + QW],
                                    start=True,
                                    stop=True,
                                )
                            # four of the 16 exp instructions per unit run
                            # on the DVE via the Schraudolph bit trick;
                            # the branch alternates per kp and unit so
                            # each softmax branch sees the same noise
                            off = None
                            if kp in (1, 3, 5, 7):
                                off = (uidx + kp // 2) % 2
                            ex = expp.tile([P, 2, QW], bf16, tag="ex")
                            if off == 0:
                                nc.vector.tensor_scalar(
                                    ex[:].bitcast(i16), sc[:],
                                    -SCHRA, SCHRB, AluMult, AluAdd)
                            else:
                                nc.scalar.activation(ex[:], sc[:], Exp,
                                                     scale=-0.125)
                            if off == 1:
                                nc.vector.tensor_scalar(
                                    exn[kp][:].bitcast(i16),
                                    sc[:], SCHRA, SCHRB, AluMult, AluAdd)
                            else:
                                nc.scalar.activation(exn[kp][:],
                                                     sc[:], Exp, scale=0.125)
                            pv_queue.append(mk_pv(kp, ex))
                            pv_pump()
                            # no eager v prefetch: the lag-2 PV closures
                            # pull v chunks on demand, so their DMA waits
                            # overlap the exp stream instead of blocking
                            # the next score pair in unit 0.  kT pulls
                            # look TWO kps ahead: the projection lump for
                            # the next chunk lands well before the score
                            # pair that depends on it.
                            require(b, "kT", min(k0 + 5, KT - 1) // 4)
                            if kp in (2, 5):
                                drain_norm(1)
                            # one bg pop per segment: two qk-projection
                            # markers in one kp is a 3.4us PE lump in
                            # front of the next score pair; backlog has
                            # 128 segments of capacity for ~100 items,
                            # so the second pop fires only under real
                            # pressure
                            if bg_queue:
                                drain_bg(1)
                            if bg_queue and len(bg_queue) > 14:
                                drain_bg(1)
                            if (uidx >= 8 and not is_last
                                    and prev_sc[0] is not None):
                                # heartbeat: keep PE duty above the HAM
                                # throttle threshold in the b1 half where
                                # no feed work remains
                                filler_into(prev_sc[0][:, 0, :], 1)
                            prev_sc[0] = sc
                        pv_queue.append(mk_finish())
                        if inject and (h, qc) in inject:
                            # finish this batch's own feed first: a later
                            # feed item reading an xst buffer an injected
                            # x-load has re-claimed would deadlock the
                            # in-order engine streams
                            while feeds[b]:
                                feed_pop(b)
                            bg_queue.extend(inject.pop((h, qc)))

            # ---------- emission ----------
            qT0, kT0, vt0 = make_feed(0)
            qT1, kT1, vt1 = make_feed(1)
            require(0, "x", 0)      # x0 DMA first in the queue
            warmup_pe()             # HAM warm by the time x0 lands
            require(0, "kT", 0)
            require(0, "qT", 0)

            oTs0 = [otsp.tile([P, S], bf16, tag=f"oTs{br}", name=f"oTs{br}_0")
                    for br in range(2)]
            oTs1 = [otsp.tile([P, S], bf16, tag=f"oTs{br}", name=f"oTs{br}_1")
                    for br in range(2)]

            # batch-1's projection feed mostly runs inside batch-1's own
            # attention (keeps PE duty balanced across both halves so the
            # HAM never throttles); only the first two chunks prefetch
            # via markers injected late in b0's attention.
            # batch-1's entire feed spreads across b0's mid/late units as
            # bg markers: the DMAs and projection matmuls double as the
            # PE-duty filler for b0, and b1 starts with everything ready.
            n1 = len(feeds[1])
            inj = {}
            for key, cnt in (((0, 1), 8), ((0, 2), 10), ((0, 3), 10)):
                take = min(cnt, n1)
                inj[key] = [feed_marker(1) for _ in range(take)]
                n1 -= take
            inj[(1, 3)] = [feed_marker(1) for _ in range(n1)]
            attention(0, qT0, kT0, vt0, oTs0, inject=inj)
            attention(1, qT1, kT1, vt1, oTs1, last=True)

            # tail: flush the pipeline (last PV groups + finish), then
            # norm chains + final outproj half-chunks.  Fillers between
            # outproj pops keep the PE clock warm while the casts/DMAs
            # trickle out.
            pv_pump(limit=0)
            assert not pending
            drain_norm(len(norm_queue))
            ti = 0
            while bg_queue:
                drain_bg(1)
                if ti % 2 == 0:
                    filler_into(fill_state["tail"][:], 1)
                ti += 1
    _split_sync_waits(nc)
    return nc


def _get_nc():
    if "nc" not in _compiled:
        _compiled["nc"] = _build()
    return _compiled["nc"]


def _prep_in_maps(x, Wq, bq, Wk, bk, Wv, bv, Wo, bo):
    ET = D // P
    xf = np.ascontiguousarray(x.reshape(B * S, D))
    # x^T tiled chunk-contiguous: [chunk, p, et, tok512], e = et*128 + p;
    # each 512-token chunk is one dense 8KB-per-partition DMA
    xt = np.ascontiguousarray(
        xf.T.reshape(ET, P, (B * S) // 512, 512).transpose(2, 1, 0, 3)
    ).astype(BF16)
    in_maps = []
    for c in range(NCORES):
        sl = slice(DSL * c, DSL * (c + 1))
        wqt = np.ascontiguousarray(
            Wq[sl].T.reshape(ET, P, DSL).transpose(1, 0, 2)
        ).astype(BF16)
        wkt = np.ascontiguousarray(
            Wk[sl].T.reshape(ET, P, DSL).transpose(1, 0, 2)
        ).astype(BF16)
        wvt = np.ascontiguousarray(
            Wv[sl].T.reshape(ET, P, DSL).transpose(1, 0, 2)
        ).astype(BF16)
        wot = np.ascontiguousarray(Wo[:, sl].T).astype(BF16)
        in_maps.append(
            {
                "xt": xt,
                "wq": wqt,
                "wk": wkt,
                "wv": wvt,
                "wo": wot,
                "bqk": np.ascontiguousarray(
                    np.stack([bq[sl], bk[sl]], axis=1)
                ).astype(np.float32),
            }
        )
    return in_maps


def kernel(x, Wq, bq, Wk, bk, Wv, bv, Wo, bo, _trace=False, _tmpdir=None):
    from concourse.bass_utils import run_bass_kernel_spmd

    x, Wq, bq, Wk, bk, Wv, bv, Wo, bo = (
        np.asarray(a, dtype=np.float32)
        for a in (x, Wq, bq, Wk, bk, Wv, bv, Wo, bo)
    )
    nc = _get_nc()
    in_maps = _prep_in_maps(x, Wq, bq, Wk, bk, Wv, bv, Wo, bo)
    res = run_bass_kernel_spmd(
        nc, in_maps, core_ids=list(range(NCORES)), trace=_trace, tmpdir=_tmpdir
    )
    total = np.zeros((2, B, S, D), np.float32)
    for c in range(NCORES):
        total += np.asarray(res.results[c]["out"], dtype=np.float32)
    const_vec = (bv @ Wo.T + bo).astype(np.float32)
    out = total[0] + const_vec
    out_comp = total[1] + const_vec
    if _trace:
        kernel._last_result = res
    return (out, out_comp)



# revision 66
# speedup vs baseline: 1.0355x; 1.0355x over previous
"""Trainium2 Bass kernel for nn_CausalAttention_5815385719336.

Dual-softmax attention: out = softmax(-QK^T/8) V Wo^T (+bias folds),
out_comp = softmax(+QK^T/8) V Wo^T.  B=2, S=2048, D=1024, H=16, DK=64.

Sharding (8 cores): Megatron-style head parallel.  Core c owns heads
(2c, 2c+1) = output dims [128c, 128c+128) of the QKV projections.  Each
core computes its head slice of Q/K/V for both batches, the full [S,S]
attention for its 4 (b, head) units (both softmax branches), and a
partial output projection o_slice @ Wo_slice^T.  The host sums the 8
partial outputs and adds the bias fold (bv @ Wo^T + bo).

On-device dataflow is fully "transposed": the host ships x^T (and W^T
slices) so every matmul contracts along partitions with zero on-device
transposes.  Scores are built as scores^T [k, q]; exp runs on the
scalar engine straight out of PSUM; P^T @ V needs no transpose.

The exp stream is the roofline.  Four of each unit's 16 FD-1024 exp
instructions run on the DVE as a Schraudolph bit-trick
(int16(A*s+B) bitcast to bf16, C tuned so the ~1.8% rms log error is
zero-mean; both branches see the same noise), cutting the scalar
engine from ~291us to ~225us of ACTIVATEs.  Scheduling:
 - score tiles are [128,2,512] PSUM pairs in a 2-deep ring; one FD-1024
   exp covers a kt pair so ACT pays its ~300-cycle overhead half as
   often;
 - a kernel-wide software pipeline (pv_queue) runs every PV/pending
   matmul two segments behind its scores/exps, flowing across unit
   boundaries: all PE work after a segment's two score matmuls is
   ready when it issues, so feed/outproj lumps amortize instead of
   starving ACT;
 - PV accumulators ([65,512], denominators riding as ones-columns of
   V) allocate lazily inside the pipeline closures so the 2-bank ring
   sequences each alloc right after the copy that frees its
   predecessor;
 - batch-0's projection feed runs up front; batch-1's entire feed
   spreads across b0's mid/late units as bg markers, doubling as the
   PE-duty filler that keeps the HAM at full clock; explicit warm
   fillers (targets chosen so they never wait a cross-engine
   semaphore) cover the b1 half and the tail;
 - the final unit runs its second branch eagerly and the tail outproj
   half-chunks borrow the idle score ring, leaving only two norm
   chains + 16 half-chunks after the last exp.
"""

import numpy as np
import ml_dtypes

B, S, D, H, DK = 2, 2048, 1024, 16, 64
NCORES = 8
HPC = H // NCORES          # heads per core = 2
DSL = HPC * DK             # d-slice per core = 128
P = 128
BF16 = ml_dtypes.bfloat16

_compiled = {}


def _install_drain_split():
    """walrus in this container rejects >1 sync wait on the Tile tail
    Drain; split extra waits into standalone wait_ge instructions."""
    import concourse.tile as tile
    from concourse.vector_clock import ScopedClock

    if getattr(tile.TileContext, "_drain_split_installed", False):
        return

    def _drain_and_barrier(self, tick_clock, wait_clock):
        nc = self.nc
        drain_inst = nc.sync.drain()
        wait_clock.add_sem_waits(
            drain_inst.ins, ScopedClock({None: tick_clock.global_clock})
        )
        si = drain_inst.ins.sync_info
        if si is not None and si.on_wait and len(si.on_wait) > 1:
            waits = list(si.on_wait)
            handles = {h.num: h for h in self.sems.allocated().values()}
            si.on_wait = waits[:1]
            for w in waits[1:]:
                assert w.wait_mode == "sem-ge-imm", w.wait_mode
                nc.sync.wait_ge(handles[w.id], w.wait_value)
        nc.all_engine_barrier()
        popped = nc._tile_sem_poison_stack.pop()
        assert popped is self._sem_poison
        nc.clear_and_free_semaphores(list(self.sems.allocated().values()))
        nc.all_engine_barrier()

    tile.TileContext._drain_and_barrier = _drain_and_barrier
    tile.TileContext._drain_split_installed = True


def _split_sync_waits(nc, max_waits=1):
    """walrus in this container has a small per-instruction sync-wait
    capacity.  Hoist excess waits onto standalone EventSemaphore
    instructions inserted just before the owner on the same engine —
    program order within an engine keeps the semantics identical."""
    from concourse import mybir

    n = 0
    for bb in nc.main_func.blocks:
        out = []
        for ins in bb.instructions:
            si = ins.sync_info
            if si is not None and si.on_wait and len(si.on_wait) > max_waits:
                waits = list(si.on_wait)
                for w in waits[:-max_waits]:
                    wi = mybir.InstEventSemaphore(name=f"W-split-{n}", ins=[], outs=[])
                    n += 1
                    wi.engine = ins.engine
                    wi.sync_info = mybir.SyncInfo(on_wait=[w], on_update=[])
                    out.append(wi)
                si.on_wait = waits[-max_waits:]
            out.append(ins)
        if n:
            bb.instructions = out


def _build():
    import concourse.bass as bass
    import concourse.tile as tile
    from concourse import mybir

    _install_drain_split()

    f32 = mybir.dt.float32
    bf16 = mybir.dt.bfloat16
    i16 = mybir.dt.int16
    Exp = mybir.ActivationFunctionType.Exp
    NT = B * S                      # 4096 tokens
    ET = D // P                     # 8 e-tiles
    # Schraudolph bit-trick exp on the DVE: int16(SCHRA*s + SCHRB)
    # reinterpreted as bf16 equals exp(s/8)*(1+eps), |eps|<~5%, zero-mean
    # (C=7.3 centers the log error; HW convert is round-to-nearest,
    # verified by probe).  Offloading 2 of 16 exp instructions per
    # attention unit to the idle DVE shaves the ACT roofline.
    SCHRA = float(0.125 * (2.0**7) / np.log(2.0))
    SCHRB = float(127 * 2**7 - 7.3)
    AluMult = mybir.AluOpType.mult
    AluAdd = mybir.AluOpType.add

    nc = bass.Bass()
    NCH = NT // 512            # 8 token chunks of 512
    xt_d = nc.declare_dram_parameter("xt", [NCH, P, ET, 512], bf16, isOutput=False)
    wq_d = nc.declare_dram_parameter("wq", [P, ET, DSL], bf16, isOutput=False)
    wk_d = nc.declare_dram_parameter("wk", [P, ET, DSL], bf16, isOutput=False)
    wv_d = nc.declare_dram_parameter("wv", [P, ET, DSL], bf16, isOutput=False)
    wo_d = nc.declare_dram_parameter("wo", [P, D], bf16, isOutput=False)
    bqk_d = nc.declare_dram_parameter("bqk", [P, 2], f32, isOutput=False)
    out_d = nc.declare_dram_parameter("out", [2, B, S, D], bf16, isOutput=True)

    KT = S // P                     # 16 k-tiles per batch
    TT = S // P                     # 16 token-tiles per batch
    QC = 4                          # q chunks per batch
    QW = S // QC                    # 512

    with tile.TileContext(nc) as tc:
        with (
            tc.tile_pool(name="singles", bufs=1) as singles,
            tc.tile_pool(name="xst", bufs=4) as xst,
            tc.tile_pool(name="perb", bufs=2) as perb,
            tc.tile_pool(name="stash", bufs=16) as stash,
            # 8-deep ex ring: a 4-deep ring made ACT exps inherit WAW
            # waits on delayed DVE Schraudolph writes when reclaiming a
            # buffer; 8 kps of distance decouples the engines fully
            tc.tile_pool(name="expp", bufs=8) as expp,
            tc.tile_pool(name="otsp", bufs=2) as otsp,
            tc.tile_pool(name="normp", bufs=4) as normp,
            tc.tile_pool(name="outp", bufs=4) as outp,
            # 8 PSUM banks: ps_sc 2x[128,2,512] = 4 (score kt-pair ring,
            # nothing else ever allocates here; one FD=1024 exp covers a
            # pair so ACT pays its ~290ns per-instruction overhead half
            # as often), ps_acc 2x[128,512] = 2 (PV pos + pending neg),
            # ps_chunk 2x[128,512] = 2 (projection / outproj /
            # denominator-broadcast chunks).
            tc.tile_pool(name="ps_sc", bufs=2, space="PSUM") as ps_sc,
            tc.tile_pool(name="ps_acc", bufs=2, space="PSUM") as ps_acc,
            tc.tile_pool(name="ps_chunk", bufs=2, space="PSUM") as ps_chunk,
        ):
            # weight tiles are allocated here but their DMAs ride inside
            # the projection feed, AFTER x0 — the 1MB x chunk is the
            # first-score critical path, the weights are small.
            wk = singles.tile([P, ET, DSL], bf16)
            wq = singles.tile([P, ET, DSL], bf16)
            bqk = singles.tile([P, 2], f32)
            bq = bqk[:, 0:1]
            bk = bqk[:, 1:2]
            wv = singles.tile([P, ET, DSL], bf16)
            wo = singles.tile([P, D], bf16)
            warm = singles.tile([P, 512], bf16)
            nc.gpsimd.memset(warm[:], 0.0)
            ones_sb = singles.tile([P, 64], bf16)
            nc.vector.memset(ones_sb[:], 1.0)

            def load_wk():
                # weight DMAs are ordered so the first-score critical
                # path (x0+wk+biases, then wq) owns the HBM stream and
                # the Sync engine's ~0.65us-per-push budget
                nc.sync.dma_start(wk[:], wk_d[:])
                nc.sync.dma_start(bqk[:], bqk_d[:])

            def load_wq():
                nc.sync.dma_start(wq[:], wq_d[:])

            def load_wv_wo():
                nc.sync.dma_start(wv[:], wv_d[:])
                nc.sync.dma_start(wo[:], wo_d[:])

            def warmup_pe():
                # ~3.5us of throwaway matmuls bridging the initial x DMA
                # wait: trips the PE HAM to K=8/8 (2.4 GHz, needs ~3.4us of
                # sustained busy) so the first projection matmuls run warm.
                ps_w = ps_chunk.tile([P, 512], f32, tag="chunk", name="warm")
                for _ in range(15):
                    nc.tensor.matmul(ps_w[:, 0:256], warm[:, 0:128],
                                     warm[:, 0:256], start=True, stop=True)

            fill_state = {}

            def filler_into(ap, n):
                # HAM-warming throwaway matmuls.  The target must be a
                # psum region whose readers have already retired (the
                # previous kp's exp'd score tile, or a dead acc bank):
                # a filler that waits on a cross-engine semaphore stalls
                # the in-order PE stream and starves ACT instead of
                # helping it.
                for _ in range(n):
                    nc.tensor.matmul(ap, warm[:, 0:128], warm[:],
                                     start=True, stop=True)

            # ---------- background queues ----------
            bg_queue = []

            def drain_bg(n=1):
                for _ in range(n):
                    if not bg_queue:
                        return
                    bg_queue.pop(0)()

            norm_queue = []

            def drain_norm(n=1):
                for _ in range(n):
                    if not norm_queue:
                        return
                    norm_queue.pop(0)()

            pending = {}

            def emit_pending_mms(k0, k1):
                if not pending:
                    return
                exn, accn, vtp, vlo, vhi = (pending[k] for k in
                                            ("exn", "acc", "vt", "vlo",
                                             "vhi"))
                for kt in range(k0, k1):
                    nc.tensor.matmul(
                        accn[0:65, :],
                        vtp[:, kt, vlo:vhi],
                        exn[kt // 2][:, kt % 2, :],
                        start=(kt == 0),
                        stop=(kt == KT - 1),
                    )

            def defer_norm(acc, b, oTs, br, h, q0, name):
                """Copy acc out of PSUM now (f32, frees its bank),
                reciprocal the denominator row on the DVE now (spread
                across 64 lanes via a tiny DMA reshape), and defer the
                ones-matmul broadcast + multiply into norm_queue.
                head1's V rows shift to partitions 64:128 via a small
                DMA so the output projection contracts both heads in
                one matmul.  The h==1 stage also emits output-projection
                half-chunks once its oTs columns are complete."""
                hp = 64 * h
                oTuD = normp.tile([P, QW], f32, tag="oTuD", name=f"oTuD{name}")
                nc.vector.tensor_copy(oTuD[0:65, :], acc[0:65, :])
                rsh = normp.tile([64, 8], f32, tag="rsh", name=f"rsh{name}")
                nc.sync.dma_start(rsh[:], oTuD[64:65, :])
                rshr = normp.tile([64, 8], bf16, tag="rshr", name=f"rshr{name}")
                with nc.allow_low_precision(reason="1/denom to bf16"):
                    nc.vector.reciprocal(rshr[:], rsh[:])
                rcp = normp.tile([P, QW], bf16, tag="rcp", name=f"rcp{name}")
                nc.sync.dma_start(rcp[64:65, :], rshr[:])
                if h == 1:
                    oTuD2 = normp.tile([P, QW], f32, tag="oTuD2",
                                       name=f"oTuD2{name}")
                    nc.sync.dma_start(oTuD2[64:128, :], oTuD[0:64, :])
                    oTuD = oTuD2

                def stage():
                    bc = ps_chunk.tile([P, QW], f32, tag="chunk",
                                       name=f"bc{name}")
                    nc.tensor.matmul(
                        bc[hp : hp + 64, :],
                        ones_sb[64:65, :],
                        rcp[64:65, :],
                        start=True,
                        stop=True,
                    )
                    nc.vector.tensor_mul(
                        oTs[br][hp : hp + 64, q0 : q0 + QW],
                        oTuD[hp : hp + 64, :],
                        bc[hp : hp + 64, :],
                    )
                    if h == 1:
                        qc = q0 // QW
                        bg_queue.extend(
                            outproj_chunks(b, oTs, br,
                                           range(qc * 4, qc * 4 + 4),
                                           alt=(b == 1 and qc == QC - 1))
                        )

                norm_queue.append(stage)

            def finish_pending():
                if not pending:
                    return
                defer_norm(pending["acc"], pending["b"], pending["oTs"], 1,
                           pending["h"], pending["q0"], pending["name"] + "n")
                pending.clear()

            # ---------- projections (progressive feed) ----------
            feeds = {}
            feed_state = {}

            def make_feed(b):
                t0 = b * S
                qT = perb.tile([P, S], bf16, tag="qT", name=f"qT_{b}")
                kT = perb.tile([P, S], bf16, tag="kT", name=f"kT_{b}")
                # vt columns: 0:64 = V head0, 64 = ones (head0 denom),
                # 65:129 = V head1, 129 = ones (head1 denom); each
                # head's PV stationary is a 65-col slice -> acc rows
                # 0:65 with the denominator at row 64.
                vt = perb.tile([P, TT, 130], bf16, tag="vt", name=f"vt_{b}")
                nc.vector.memset(vt[:, :, 64], 1.0)
                nc.vector.memset(vt[:, :, 129], 1.0)
                cell = {}

                def load_chunk(xc, split=False):
                    def go():
                        xtile = xst.tile([P, ET, 512], bf16, tag="xtile",
                                         name=f"xt_{b}_{xc}")
                        gc = b * QC + xc
                        if split:
                            # two parallel DMAs halve the critical-path
                            # latency of the very first x chunk
                            nc.sync.dma_start(xtile[:, 0:4, :],
                                              xt_d[gc, :, 0:4, :])
                            nc.sync.dma_start(xtile[:, 4:8, :],
                                              xt_d[gc, :, 4:8, :])
                        else:
                            nc.sync.dma_start(xtile[:], xt_d[gc])
                        cell[xc] = xtile
                    return go

                def qk_chunk(xc, w_t, bias_t, dst, half):
                    # half 0 emits the first 4 e-tiles, half 1 the last 4
                    # plus the bias/copy-out: each feed pop stays <1us of
                    # PE so attention score matmuls never wait long.
                    def go():
                        xtile = cell[xc]
                        key = ("pj", xc, id(w_t))
                        if half == 0:
                            ps = ps_chunk.tile([P, 512], f32, tag="chunk",
                                               name=f"pj_{b}_{xc}_{id(w_t)}")
                            cell[key] = ps
                        else:
                            ps = cell[key]
                        for et in range(4 * half, 4 * half + 4):
                            nc.tensor.matmul(
                                ps, w_t[:, et, :], xtile[:, et, :],
                                start=(et == 0), stop=(et == ET - 1),
                            )
                        if half == 1:
                            nc.vector.tensor_scalar_add(
                                dst[:, xc * 512 : (xc + 1) * 512], ps, bias_t
                            )
                    return go

                def v_chunk(xc, vtt):
                    def go():
                        xtile = cell[xc]
                        tt = xc * 4 + vtt
                        pv = ps_chunk.tile([P, DSL], f32, tag="chunk",
                                           name=f"pv_{b}_{tt}")
                        for et in range(ET):
                            nc.tensor.matmul(
                                pv, xtile[:, et, vtt * P : (vtt + 1) * P],
                                wv[:, et, :],
                                start=(et == 0), stop=(et == ET - 1),
                            )
                        nc.vector.tensor_copy(vt[:, tt, 0:64], pv[:, 0:64])
                        nc.vector.tensor_copy(vt[:, tt, 65:129],
                                              pv[:, 64:128])
                    return go

                if b == 0:
                    items = [
                        ("x", 0, load_chunk(0, split=True)),
                        ("w", 0, load_wk),
                        ("kTh", 0, qk_chunk(0, wk, bk, kT, 0)),
                        ("kT", 0, qk_chunk(0, wk, bk, kT, 1)),
                        ("w", 1, load_wq),
                        ("qTh", 0, qk_chunk(0, wq, bq, qT, 0)),
                        ("qT", 0, qk_chunk(0, wq, bq, qT, 1)),
                        ("w", 2, load_wv_wo),
                        ("x", 1, load_chunk(1)),
                        ("x", 2, load_chunk(2)),
                        ("x", 3, load_chunk(3)),
                        ("v", 0, v_chunk(0, 0)),
                        ("v", 1, v_chunk(0, 1)),
                        ("v", 2, v_chunk(0, 2)),
                        ("v", 3, v_chunk(0, 3)),
                        ("kTh", 1, qk_chunk(1, wk, bk, kT, 0)),
                        ("kT", 1, qk_chunk(1, wk, bk, kT, 1)),
                        ("qTh", 1, qk_chunk(1, wq, bq, qT, 0)),
                        ("qT", 1, qk_chunk(1, wq, bq, qT, 1)),
                    ]
                else:
                    # batch 1: all x loads issue up front (the DMAs
                    # overlap late-b0 compute; xst is 4 deep)
                    items = [
                        ("x", 0, load_chunk(0)),
                        ("x", 1, load_chunk(1)),
                        ("x", 2, load_chunk(2)),
                        ("x", 3, load_chunk(3)),
                        ("kTh", 0, qk_chunk(0, wk, bk, kT, 0)),
                        ("kT", 0, qk_chunk(0, wk, bk, kT, 1)),
                        ("qTh", 0, qk_chunk(0, wq, bq, qT, 0)),
                        ("qT", 0, qk_chunk(0, wq, bq, qT, 1)),
                        ("v", 0, v_chunk(0, 0)),
                        ("v", 1, v_chunk(0, 1)),
                        ("v", 2, v_chunk(0, 2)),
                        ("v", 3, v_chunk(0, 3)),
                        ("kTh", 1, qk_chunk(1, wk, bk, kT, 0)),
                        ("kT", 1, qk_chunk(1, wk, bk, kT, 1)),
                        ("qTh", 1, qk_chunk(1, wq, bq, qT, 0)),
                        ("qT", 1, qk_chunk(1, wq, bq, qT, 1)),
                    ]
                items += [
                    ("v", 4, v_chunk(1, 0)),
                    ("v", 5, v_chunk(1, 1)),
                    ("v", 6, v_chunk(1, 2)),
                    ("v", 7, v_chunk(1, 3)),
                    ("kTh", 2, qk_chunk(2, wk, bk, kT, 0)),
                    ("kT", 2, qk_chunk(2, wk, bk, kT, 1)),
                    ("qTh", 2, qk_chunk(2, wq, bq, qT, 0)),
                    ("qT", 2, qk_chunk(2, wq, bq, qT, 1)),
                    ("v", 8, v_chunk(2, 0)),
                    ("v", 9, v_chunk(2, 1)),
                    ("v", 10, v_chunk(2, 2)),
                    ("v", 11, v_chunk(2, 3)),
                    ("kTh", 3, qk_chunk(3, wk, bk, kT, 0)),
                    ("kT", 3, qk_chunk(3, wk, bk, kT, 1)),
                    ("qTh", 3, qk_chunk(3, wq, bq, qT, 0)),
                    ("qT", 3, qk_chunk(3, wq, bq, qT, 1)),
                    ("v", 12, v_chunk(3, 0)),
                    ("v", 13, v_chunk(3, 1)),
                    ("v", 14, v_chunk(3, 2)),
                    ("v", 15, v_chunk(3, 3)),
                ]
                feeds[b] = items
                feed_state[b] = {"x": -1, "kT": -1, "qT": -1, "v": -1,
                                 "w": -1, "kTh": -1, "qTh": -1}
                return qT, kT, vt

            def feed_pop(b):
                if not feeds[b]:
                    return False
                label, idx, go = feeds[b].pop(0)
                go()
                feed_state[b][label] = idx
                return True

            def require(b, label, idx):
                while feed_state[b][label] < idx:
                    assert feed_pop(b), (b, label, idx)

            def feed_marker(b):
                def go():
                    feed_pop(b)
                return go

            def outproj_chunks(b, oTs, br, tts, alt=False):
                """Output-projection half-chunk closures for (b, br,
                tts): one [128,512] matmul + cast + DMA per (tt, oc).
                alt=True marks tail chunks: casts alternate between DVE
                and ACT (ACT has gone idle), and the po tiles come from
                the score-ring PSUM pool (idle at the tail) so several
                matmuls can be in flight ahead of the casts."""
                chunks = []
                cellp = {}

                def one(tt, oc, cast_eng):
                    def go():
                        if alt:
                            if tt not in cellp:
                                cellp[tt] = ps_sc.tile(
                                    [P, 2, QW], f32, tag="sc",
                                    name=f"pot_{b}_{br}_{tt}")
                            po = cellp[tt][:, oc, :]
                        else:
                            po = ps_chunk.tile([P, 512], f32, tag="chunk",
                                               name=f"po_{b}_{br}_{tt}_{oc}")
                        nc.tensor.matmul(
                            po,
                            oTs[br][:, tt * P : (tt + 1) * P],
                            wo[:, oc * 512 : (oc + 1) * 512],
                            start=True,
                            stop=True,
                        )
                        ob = outp.tile([P, 512], bf16, tag="ob")
                        cast_eng(ob[:], po)
                        nc.sync.dma_start(
                            out_d[br, b, tt * P : (tt + 1) * P,
                                  oc * 512 : (oc + 1) * 512], ob[:]
                        )
                    return go

                i = 0
                for tt in tts:
                    for oc in range(2):
                        # casts alternate ACT/DVE everywhere: with four
                        # exps per unit offloaded to the DVE, ACT has
                        # exactly the slack for half the casts — keeping
                        # its queue full instead of idling, while the
                        # lighter DVE queue returns the Schraudolph
                        # exps well inside the score-ring slack.
                        eng = (nc.scalar.copy if (alt and i % 2 == 1)
                               else nc.vector.tensor_copy)
                        chunks.append(one(tt, oc, eng))
                        i += 1
                return chunks

            # ---------- attention ----------
            # The kernel-wide software pipeline: scores+exps for a kp
            # issue first; the matching PV/pending matmuls are queued as
            # closures and pop TWO segments later (flowing across unit
            # boundaries), so every PE matmul after the two score mms at
            # a segment's head is ready the moment it issues — feed and
            # outproj lumps amortize inside the two-segment slack instead
            # of delaying the next score pair and starving ACT.
            unit_ctr = [0]
            pv_queue = []

            def pv_pump(limit=2):
                while len(pv_queue) > limit:
                    pv_queue.pop(0)()

            def attention(b, qT, kT, vt, oTs, inject=None, last=False):
                # qc outer / h inner: each (h1, qc) norm completes a
                # quarter-batch of output columns mid-attention, so its
                # output-projection halves drain before the tail.
                prev_sc = [None]
                for qc in range(QC):
                    q0 = qc * QW
                    for h in range(HPC):
                        hp = 64 * h
                        vlo, vhi = (0, 65) if h == 0 else (65, 130)
                        require(b, "qT", qc)
                        name = f"_{b}_{h}_{qc}"
                        is_last = last and qc == QC - 1 and h == HPC - 1
                        uidx = unit_ctr[0]
                        unit_ctr[0] += 1
                        # per-kp exn tiles: the deferred branch's exps
                        # come from two engines (ACT and the DVE bit
                        # trick); separate tiles keep each one single-
                        # writer so no cross-engine write-order coupling
                        # ever delays the exp stream
                        exn = [stash.tile([P, 2, QW], bf16, tag="exn",
                                          name=f"exn{name}_{kp}")
                               for kp in range(KT // 2)]
                        # acc banks allocate lazily inside the closures:
                        # the 2-deep psum ring only works because each
                        # accumulator is claimed right after the copy
                        # that frees its predecessor has been emitted.
                        ust = {"acc": None, "accn": None}

                        def mk_pv(kp, ex, ust=ust, vt=vt, vlo=vlo,
                                  vhi=vhi, b=b, is_last=is_last, exn=exn,
                                  name=name):
                            def go():
                                if ust["acc"] is None:
                                    ust["acc"] = ps_acc.tile(
                                        [P, QW], f32, tag="acc",
                                        name=f"accp{name}")
                                acc = ust["acc"]
                                require(b, "v", 2 * kp + 1)
                                for j in range(2):
                                    kt = 2 * kp + j
                                    nc.tensor.matmul(
                                        acc[0:65, :],
                                        vt[:, kt, vlo:vhi],
                                        ex[:, j, :],
                                        start=(kt == 0),
                                        stop=(kt == KT - 1),
                                    )
                                    if not is_last:
                                        emit_pending_mms(kt, kt + 1)
                                if is_last:
                                    # final unit: drain the previous
                                    # unit's deferred branch at 4 mms per
                                    # segment so its acc bank frees
                                    # mid-loop, then accumulate this
                                    # unit's own second branch eagerly.
                                    if kp < 4:
                                        emit_pending_mms(4 * kp, 4 * kp + 4)
                                        if kp == 3:
                                            finish_pending()
                                    else:
                                        if ust["accn"] is None:
                                            ust["accn"] = ps_acc.tile(
                                                [P, QW], f32, tag="acc",
                                                name=f"accn{name}")
                                        lo, hi = {4: (0, 6), 5: (6, 10),
                                                  6: (10, 14),
                                                  7: (14, 16)}[kp]
                                        for kte in range(lo, hi):
                                            nc.tensor.matmul(
                                                ust["accn"][0:65, :],
                                                vt[:, kte, vlo:vhi],
                                                exn[kte // 2][:, kte % 2, :],
                                                start=(kte == 0),
                                                stop=(kte == KT - 1),
                                            )
                            return go

                        def mk_finish(ust=ust, b=b, oTs=oTs, h=h, q0=q0,
                                      name=name, is_last=is_last, vt=vt,
                                      vlo=vlo, vhi=vhi, exn=exn):
                            def go():
                                if is_last:
                                    defer_norm(ust["acc"], b, oTs, 0, h,
                                               q0, name + "p")
                                    defer_norm(ust["accn"], b, oTs, 1, h,
                                               q0, name + "n")
                                    tail_fill = ps_acc.tile(
                                        [P, QW], f32, tag="acc",
                                        name="tailfill")
                                    fill_state["tail"] = tail_fill
                                    filler_into(tail_fill[:], 10)
                                    return
                                finish_pending()
                                defer_norm(ust["acc"], b, oTs, 0, h, q0,
                                           name + "p")
                                accn = ps_acc.tile([P, QW], f32, tag="acc",
                                                   name=f"accn{name}")
                                pending.update(exn=exn, acc=accn, vt=vt,
                                               vlo=vlo, vhi=vhi, oTs=oTs,
                                               b=b, h=h, q0=q0, name=name)
                            return go

                        for kp in range(KT // 2):
                            k0 = 2 * kp
                            require(b, "kT", (k0 + 1) // 4)
                            sc = ps_sc.tile([P, 2, QW], f32, tag="sc",
                                            name=f"sc{name}_{kp}")
                            for j in range(2):
                                kt = k0 + j
                                nc.tensor.matmul(
                                    sc[:, j, :],
                                    kT[hp : hp + 64, kt * P : (kt + 1) * P],
                                    qT[hp : hp + 64, q0 : q0 + QW],
                                    start=True,
                                    stop=True,
                                )
                            # four of the 16 exp instructions per unit run
                            # on the DVE via the Schraudolph bit trick;
                            # the branch alternates per kp and unit so
                            # each softmax branch sees the same noise
                            off = None
                            if kp in (1, 3, 5, 7):
                                off = (uidx + kp // 2) % 2
                            ex = expp.tile([P, 2, QW], bf16, tag="ex")
                            if off == 0:
                                nc.vector.tensor_scalar(
                                    ex[:].bitcast(i16), sc[:],
                                    -SCHRA, SCHRB, AluMult, AluAdd)
                            else:
                                nc.scalar.activation(ex[:], sc[:], Exp,
                                                     scale=-0.125)
                            if off == 1:
                                nc.vector.tensor_scalar(
                                    exn[kp][:].bitcast(i16),
                                    sc[:], SCHRA, SCHRB, AluMult, AluAdd)
                            else:
                                nc.scalar.activation(exn[kp][:],
                                                     sc[:], Exp, scale=0.125)
                            pv_queue.append(mk_pv(kp, ex))
                            pv_pump()
                            # no eager v prefetch: the lag-2 PV closures
                            # pull v chunks on demand, so their DMA waits
                            # overlap the exp stream instead of blocking
                            # the next score pair in unit 0.  kT pulls
                            # look TWO kps ahead: the projection lump for
                            # the next chunk lands well before the score
                            # pair that depends on it.
                            require(b, "kT", min(k0 + 5, KT - 1) // 4)
                            if kp in (2, 5):
                                drain_norm(1)
                            if bg_queue and (kp % 1 == 0 or len(bg_queue) > 6):
                                drain_bg(1)
                            if bg_queue and len(bg_queue) > 6:
                                drain_bg(1)
                            if (uidx >= 8 and not is_last
                                    and prev_sc[0] is not None):
                                # heartbeat: keep PE duty above the HAM
                                # throttle threshold in the b1 half where
                                # no feed work remains
                                filler_into(prev_sc[0][:, 0, :], 1)
                            prev_sc[0] = sc
                        pv_queue.append(mk_finish())
                        if inject and (h, qc) in inject:
                            # finish this batch's own feed first: a later
                            # feed item reading an xst buffer an injected
                            # x-load has re-claimed would deadlock the
                            # in-order engine streams
                            while feeds[b]:
                                feed_pop(b)
                            bg_queue.extend(inject.pop((h, qc)))

            # ---------- emission ----------
            qT0, kT0, vt0 = make_feed(0)
            qT1, kT1, vt1 = make_feed(1)
            require(0, "x", 0)      # x0 DMA first in the queue
            warmup_pe()             # HAM warm by the time x0 lands
            require(0, "kT", 0)
            require(0, "qT", 0)

            oTs0 = [otsp.tile([P, S], bf16, tag=f"oTs{br}", name=f"oTs{br}_0")
                    for br in range(2)]
            oTs1 = [otsp.tile([P, S], bf16, tag=f"oTs{br}", name=f"oTs{br}_1")
                    for br in range(2)]

            # batch-1's projection feed mostly runs inside batch-1's own
            # attention (keeps PE duty balanced across both halves so the
            # HAM never throttles); only the first two chunks prefetch
            # via markers injected late in b0's attention.
            # batch-1's entire feed spreads across b0's mid/late units as
            # bg markers: the DMAs and projection matmuls double as the
            # PE-duty filler for b0, and b1 starts with everything ready.
            n1 = len(feeds[1])
            inj = {}
            for key, cnt in (((0, 1), 8), ((0, 2), 10), ((0, 3), 10)):
                take = min(cnt, n1)
                inj[key] = [feed_marker(1) for _ in range(take)]
                n1 -= take
            inj[(1, 3)] = [feed_marker(1) for _ in range(n1)]
            attention(0, qT0, kT0, vt0, oTs0, inject=inj)
            attention(1, qT1, kT1, vt1, oTs1, last=True)

            # tail: flush the pipeline (last PV groups + finish), then
            # norm chains + final outproj half-chunks.  Fillers between
            # outproj pops keep the PE clock warm while the casts/DMAs
            # trickle out.
            pv_pump(limit=0)
            assert not pending
            drain_norm(len(norm_queue))
            ti = 0
            while bg_queue:
                drain_bg(1)
                if ti % 2 == 0:
                    filler_into(fill_state["tail"][:], 1)
                ti += 1
    _split_sync_waits(nc)
    return nc


def _get_nc():
    if "nc" not in _compiled:
        _compiled["nc"] = _build()
    return _compiled["nc"]


def _prep_in_maps(x, Wq, bq, Wk, bk, Wv, bv, Wo, bo):
    ET = D // P
    xf = np.ascontiguousarray(x.reshape(B * S, D))
    # x^T tiled chunk-contiguous: [chunk, p, et, tok512], e = et*128 + p;
    # each 512-token chunk is one dense 8KB-per-partition DMA
    xt = np.ascontiguousarray(
        xf.T.reshape(ET, P, (B * S) // 512, 512).transpose(2, 1, 0, 3)
    ).astype(BF16)
    in_maps = []
    for c in range(NCORES):
        sl = slice(DSL * c, DSL * (c + 1))
        wqt = np.ascontiguousarray(
            Wq[sl].T.reshape(ET, P, DSL).transpose(1, 0, 2)
        ).astype(BF16)
        wkt = np.ascontiguousarray(
            Wk[sl].T.reshape(ET, P, DSL).transpose(1, 0, 2)
        ).astype(BF16)
        wvt = np.ascontiguousarray(
            Wv[sl].T.reshape(ET, P, DSL).transpose(1, 0, 2)
        ).astype(BF16)
        wot = np.ascontiguousarray(Wo[:, sl].T).astype(BF16)
        in_maps.append(
            {
                "xt": xt,
                "wq": wqt,
                "wk": wkt,
                "wv": wvt,
                "wo": wot,
                "bqk": np.ascontiguousarray(
                    np.stack([bq[sl], bk[sl]], axis=1)
                ).astype(np.float32),
            }
        )
    return in_maps


def kernel(x, Wq, bq, Wk, bk, Wv, bv, Wo, bo, _trace=False, _tmpdir=None):
    from concourse.bass_utils import run_bass_kernel_spmd

    x, Wq, bq, Wk, bk, Wv, bv, Wo, bo = (
        np.asarray(a, dtype=np.float32)
        for a in (x, Wq, bq, Wk, bk, Wv, bv, Wo, bo)
    )
    nc = _get_nc()
    in_maps = _prep_in_maps(x, Wq, bq, Wk, bk, Wv, bv, Wo, bo)
    res = run_bass_kernel_spmd(
        nc, in_maps, core_ids=list(range(NCORES)), trace=_trace, tmpdir=_tmpdir
    )
    total = np.zeros((2, B, S, D), np.float32)
    for c in range(NCORES):
        total += np.asarray(res.results[c]["out"], dtype=np.float32)
    const_vec = (bv @ Wo.T + bo).astype(np.float32)
    out = total[0] + const_vec
    out_comp = total[1] + const_vec
    if _trace:
        kernel._last_result = res
    return (out, out_comp)

